# revision 45
# baseline (speedup 1.0000x reference)
"""Trainium2 Bass/Tile kernel for nn_Decoder (GRU decoder with teacher forcing).

Math (per reference):
  zx  = [enc_h_feat, z]                    (B, 1056)
  h0  = zx @ W_dh.T + b_dh                 (B, 128)
  a0  = last_obs @ W_vel.T + b_vel         (B, 2)
  rel = (sg - last_obs[:, :2]) / dt        (B, 2)
  a_t = a0 if t==0 else fut_traj[t-1,:,2:4]
  x_t = [zx, a_t, rel]  -> GRUCell(x_t, h) -> mu_t, std_t

Device strategy (8 cores, batch-sharded, 2048 rows/core), all fp16
operands with fp32 PSUM accumulation:
  - Feature-on-partition, batch-on-free layout; free chunks of 512.
  - Setup: [gi_r|gi_z|gi_n|h0](512 rows) = W_big.T @ XT with K=1065
    host-packed rows [zxT; sgT; loT; ones].  The rel term and all
    input-side biases are folded into W_big on the host.  The K=1024 zx
    block runs as fp8e4m3 DoubleRow matmuls (256 K-rows/instruction at
    0.5 cy/row) with hi+lo residual splits of both operands, 3 passes
    (hi*whi + lo*whi + hi*wlo); the 41-row tail stays fp16.  W is
    host-scaled x32 into e4m3's normal range (raw |W|~0.03 lands in
    fp8 subnormals, which cost 13x in end-to-end error) and the PSUM
    readout divides it back out.
  - Per step/chunk i=(t,c), software-pipelined one chunk deep so PE's
    in-order stream never waits on the sigmoid->q chain:
      stage A(i):  psum_rz  = Whh_{r,z}@h + K3@[a;1] + I@gi_{r,z}   [PE x6]
                   psum_hn  = Whh_n@h                               [PE x1]
                   rz       = sigmoid(psum_rz)                      [ScalarE]
                   q        = (psum_hn + b_hh_n) * r                [DVE stt]
      stage B(i-1): q2 = q + gi_n                     [DVE fp16 2x tt]
                   psum_gin = K2@a + I@q2                           [PE x2]
                   n  = tanh(psum_gin)                              [ScalarE]
                   d = h - n   [DVE 2x tt];   e = z*d       [GPSIMD]
      stage C(i-2): h' = n + e                        [DVE fp16 2x tt]
    (9 PE matmuls per step-chunk is the floor: every psum-accumulated
    term costs one 512-cycle stream regardless of K; a DVE->PSUM
    warm-start that would drop I@q2 computes wrong results on HW.)
    h' for every step is kept in SBUF (25 x 0.5MB fp16 tiles), so no
    PSUM bank is pinned during the loop: rz/hn/gin pools all run bufs=2
    and PE never stalls on a WAR against the activation reads.
  - Post-loop: per-step head matmuls with a scattered-column lhsT
    accumulate mu/std pre-activations for ALL steps into 4 PSUM tiles;
    mu = Identity(+b_mu), std = Exp(0.5*(.)+0.5*b_std) = sqrt(exp(.)).
  - Engine balance per step-chunk in the loop: PE 1.92us (bottleneck), ScalarE
    1.65us, DVE 1.64us, GPSIMD 1.1us.  Bulk DMAs issue from SP/
    Activation sequencers (HWDGE path, no engine cost); setup weights
    ride GPSIMD's SWDGE queue, which runs parallel to HWDGE.
Host does only sharding/transposes/weight packing (a0 is a (B,6)@(6,2)
matmul on host, ~0.4 MFLOP, negligible vs the 52 GFLOP kernel).
"""

import numpy as np

import concourse.bass as bass
import concourse.mybir as mybir
import concourse.tile as tile
from concourse import bacc
from concourse.bass_utils import run_bass_kernel_spmd

F32 = mybir.dt.float32
F16 = mybir.dt.float16
F8 = mybir.dt.float8e4
DRM = mybir.MatmulPerfMode.DoubleRow
AF = mybir.ActivationFunctionType
OP = mybir.AluOpType

B, T, MLP, ZD, H, NS, NP = 16384, 24, 1024, 32, 128, 6, 2
NCORES = 8
BC = B // NCORES            # 2048 rows per core
F = 512                     # free-dim chunk
NF = BC // F                # 4 chunks
KIN = MLP + ZD + NP + NS + 1  # 1065 = zx(1056) + sg(2) + lo(6) + ones(1)
NKC = (KIN + 127) // 128    # 9 K-chunks (8x128 + 41)
DT_CONST = 0.4 * 12


def build_nc(debug=False, t_steps=T):
    HD = F16
    nc = bacc.Bacc("TRN2", target_bir_lowering=False, debug=debug)

    # ---- DRAM I/O ----
    # setup operands: 4 chunks of 256 K-rows as fp8 hi/lo (DoubleRow pairs
    # rows p and p+128 of a chunk), plus a 41-row fp16 remainder
    x8h_d = nc.dram_tensor("x8h", [4, 128, 2, BC], F8, kind="ExternalInput").ap()
    x8l_d = nc.dram_tensor("x8l", [4, 128, 2, BC], F8, kind="ExternalInput").ap()
    w8h_d = nc.dram_tensor("w8h", [4, 128, 2, 512], F8, kind="ExternalInput").ap()
    w8l_d = nc.dram_tensor("w8l", [4, 128, 2, 512], F8, kind="ExternalInput").ap()
    xtr_d = nc.dram_tensor("xtr", [41, BC], F16, kind="ExternalInput").ap()
    wr_d = nc.dram_tensor("wr", [41, 512], F16, kind="ExternalInput").ap()
    a3_d = nc.dram_tensor("a3", [t_steps, 3, BC], F16, kind="ExternalInput").ap()
    whht_d = nc.dram_tensor("whht", [H, 3 * H], F16, kind="ExternalInput").ap()
    k3p_d = nc.dram_tensor("k3p", [66, H], F16, kind="ExternalInput").ap()
    _std_off = ((2 * t_steps + 31) // 32) * 32
    _m_head = _std_off + 2 * t_steps
    wmsx_d = nc.dram_tensor("wmsx", [H, t_steps * _m_head], F16,
                            kind="ExternalInput").ap()
    id_d = nc.dram_tensor("identh", [H, H], F16, kind="ExternalInput").ap()
    bhhn_d = nc.dram_tensor("bhhn", [H, 1], F32, kind="ExternalInput").ap()
    bmu_d = nc.dram_tensor("bmu48", [2 * t_steps, 1], F32, kind="ExternalInput").ap()
    bstd_d = nc.dram_tensor("bstd48", [2 * t_steps, 1], F32, kind="ExternalInput").ap()
    omu_d = nc.dram_tensor("omu", [2 * t_steps, BC], F32, kind="ExternalOutput").ap()
    ostd_d = nc.dram_tensor("ostd", [2 * t_steps, BC], F32, kind="ExternalOutput").ap()

    with tile.TileContext(nc) as tc:
        with tc.tile_pool(name="persist", bufs=1) as pp:
            # persistent SBUF state
            gi_r = pp.tile([H, BC], HD)
            gi_z = pp.tile([H, BC], HD)
            gi_n = pp.tile([H, BC], HD)
            # h history: one tile per step boundary (h_state[t] = h before
            # step t); separate tiles keep dependency tracking per-step
            hst = [pp.tile([H, BC], HD, name=f"hst{t}") for t in range(t_steps + 1)]
            std_off = ((2 * t_steps + 31) // 32) * 32
            m_head = std_off + 2 * t_steps
            whht_t = pp.tile([H, 3 * H], HD)
            k3p_t = pp.tile([66, H], HD)
            wmsx_t = pp.tile([H, t_steps * m_head], HD)
            bhhn_t = pp.tile([H, 1], F32)
            bmu_t = pp.tile([2 * t_steps, 1], F32)
            bstd_t = pp.tile([2 * t_steps, 1], F32)
            ident = pp.tile([H, H], HD)

            gi_dst = [gi_r, gi_z, gi_n, None]

            # ---- setup: [gi | h0] = W_big.T @ XT ----
            # fp8 DoubleRow 3-pass (hi*whi + lo*whi + hi*wlo): 0.5 cy/row
            # and 256 K-rows per instruction; dropped lo*wlo term ~0.4%
            with tc.tile_pool(name="xtp", bufs=1) as xtp, \
                 tc.tile_pool(name="wp", bufs=1) as wp, \
                 tc.tile_pool(name="sps", bufs=4, space="PSUM") as sps:
                x8h_t, x8l_t, w8h_t, w8l_t = [], [], [], []
                for q in range(4):
                    x8h_t.append(xtp.tile([128, 2, BC], F8, name=f"x8h{q}",
                                          tag=f"x8h{q}"))
                    x8l_t.append(xtp.tile([128, 2, BC], F8, name=f"x8l{q}",
                                          tag=f"x8l{q}"))
                    w8h_t.append(wp.tile([128, 2, 512], F8, name=f"w8h{q}",
                                         tag=f"w8h{q}"))
                    w8l_t.append(wp.tile([128, 2, 512], F8, name=f"w8l{q}",
                                         tag=f"w8l{q}"))
                xtr_t = xtp.tile([41, BC], F16, name="xtr", tag="xtr")
                wr_t = wp.tile([41, 512], F16, name="wr", tag="wr")
                for q in range(4):
                    _eng = nc.sync if q == 0 else nc.gpsimd
                    _eng.dma_start(w8h_t[q][:], w8h_d[q])
                    _eng.dma_start(w8l_t[q][:], w8l_d[q])
                nc.gpsimd.dma_start(wr_t[:], wr_d)
                # column(nci)-major x loads: the nci-outer group loop below
                # consumes column 0 while column 1 lands
                for nci in range(NF):
                    cs = slice(nci * F, (nci + 1) * F)
                    for q in range(4):
                        # hi planes gate passes 1-2, lo only pass 3: put hi
                        # and lo on different HWDGE queues so hi lands first
                        nc.scalar.dma_start(x8h_t[q][:, :, cs],
                                            x8h_d[q][:, :, cs])
                        nc.sync.dma_start(x8l_t[q][:, :, cs],
                                          x8l_d[q][:, :, cs])
                    [nc.scalar, nc.sync][nci % 2].dma_start(
                        xtr_t[:, cs], xtr_d[:, cs])
                # loop/head weights ride the Pool SWDGE queue as well --
                # it is idle during setup and bypasses the shared HWDGE
                nc.gpsimd.dma_start(ident[:], id_d)
                nc.gpsimd.dma_start(whht_t[:], whht_d)
                nc.gpsimd.dma_start(k3p_t[:], k3p_d)
                nc.gpsimd.dma_start(bhhn_t[:], bhhn_d)
                nc.gpsimd.dma_start(wmsx_t[:], wmsx_d)
                nc.gpsimd.dma_start(bmu_t[:], bmu_d)
                nc.gpsimd.dma_start(bstd_t[:], bstd_d)

                for nci in range(NF):
                    cs = slice(nci * F, (nci + 1) * F)
                    for m in range(4):
                        ms = slice(128 * m, 128 * (m + 1))
                        ps = sps.tile([128, F], F32, name="setps", tag="setps")
                        for q in range(4):
                            nc.tensor.matmul(ps[:], w8h_t[q][:, :, ms],
                                             x8h_t[q][:, :, cs],
                                             start=(q == 0), stop=False,
                                             perf_mode=DRM)
                            nc.tensor.matmul(ps[:], w8h_t[q][:, :, ms],
                                             x8l_t[q][:, :, cs],
                                             start=False, stop=False,
                                             perf_mode=DRM)
                            nc.tensor.matmul(ps[:], w8l_t[q][:, :, ms],
                                             x8h_t[q][:, :, cs],
                                             start=False, stop=False,
                                             perf_mode=DRM)
                        nc.tensor.matmul(ps[:], wr_t[:, ms], xtr_t[:, cs],
                                         start=False, stop=True)
                        dst = gi_dst[m] if m < 3 else hst[0]
                        # W was host-scaled x32 into e4m3's normal range
                        # (raw |W|~0.03 sits in fp8 subnormals); undo here
                        nc.vector.tensor_scalar_mul(dst[:, cs], ps[:],
                                                    1.0 / 32.0)

            # ---- recurrence, software-pipelined one chunk deep ----
            NI = t_steps * NF
            with tc.tile_pool(name="a3p", bufs=3) as a3p, \
                 tc.tile_pool(name="gp", bufs=3) as gp, \
                 tc.tile_pool(name="prz", bufs=2, space="PSUM") as prz, \
                 tc.tile_pool(name="phn", bufs=2, space="PSUM") as phn, \
                 tc.tile_pool(name="pgin", bufs=2, space="PSUM") as pgin:
                a3_tiles = {}
                live = {}   # flat index -> dict of tiles for stage B
                for i in range(NI + 2):
                    # ---- stage C for flat index i-2 (emitted first so the
                    # h' write clears DVE's in-order queue before stt/q2/d
                    # pile in; the next step's matmuls wait on it) ----
                    if i >= 2:
                        lv = live.pop(i - 2)
                        nc.vector.tensor_tensor(hst[lv["t"] + 1][:, lv["cs"]],
                                                lv["nt"][:], lv["e"][:], op=OP.add)
                    # ---- stage A for flat index i ----
                    if i < NI:
                        t, c = divmod(i, NF)
                        cs = slice(c * F, (c + 1) * F)
                        hs = hst[t][:, cs]
                        if c == 0:
                            a3_t = a3p.tile([66, BC], HD, name="a3t", tag="a3t")
                            nc.sync.dma_start(a3_t[0:3, :], a3_d[t])
                            nc.sync.dma_start(a3_t[32:35, :], a3_d[t])
                            nc.sync.dma_start(a3_t[64:66, :], a3_d[t][0:2])
                            a3_tiles[t] = a3_t
                        a3_t = a3_tiles[t]
                        psum_rz = prz.tile([128, 2 * F], F32, name="psrz", tag="psrz")
                        nc.tensor.matmul(psum_rz[:, 0:F], whht_t[:, 0:H], hs,
                                         start=True, stop=False)
                        nc.tensor.matmul(psum_rz[:, F:2 * F], whht_t[:, H:2 * H], hs,
                                         start=True, stop=False)
                        nc.tensor.matmul(psum_rz[:, 0:F], k3p_t[0:3, :],
                                         a3_t[0:3, cs], start=False, stop=False)
                        nc.tensor.matmul(psum_rz[:, F:2 * F], k3p_t[32:35, :],
                                         a3_t[32:35, cs], start=False, stop=False)
                        nc.tensor.matmul(psum_rz[:, 0:F], ident[:], gi_r[:, cs],
                                         start=False, stop=True)
                        nc.tensor.matmul(psum_rz[:, F:2 * F], ident[:], gi_z[:, cs],
                                         start=False, stop=True)
                        psum_hn = phn.tile([128, F], F32, name="pshn", tag="pshn")
                        nc.tensor.matmul(psum_hn[:], whht_t[:, 2 * H:3 * H], hs,
                                         start=True, stop=True)
                        rz = gp.tile([128, 2 * F], HD, name="rz", tag="rz")
                        nc.scalar.activation(rz[:], psum_rz[:], AF.Sigmoid)
                        q = gp.tile([128, F], HD, name="q", tag="q")
                        nc.vector.scalar_tensor_tensor(
                            q[:], psum_hn[:], bhhn_t[:], rz[:, 0:F],
                            op0=OP.add, op1=OP.mult,
                        )
                        live[i] = dict(t=t, cs=cs, hs=hs, rz=rz, a3=a3_t, q=q)
                    # ---- stage B for flat index i-1 ----
                    if 1 <= i <= NI:
                        lv = live[i - 1]
                        t, cs = lv["t"], lv["cs"]
                        # q2 = q + gi_n on DVE replaces a third PE inject
                        q2 = gp.tile([128, F], HD, name="q2", tag="q2")
                        nc.vector.tensor_tensor(q2[:], lv["q"][:], gi_n[:, cs],
                                                op=OP.add)
                        psum_gin = pgin.tile([128, F], F32, name="psgin",
                                             tag="psgin")
                        nc.tensor.matmul(psum_gin[:], k3p_t[64:66, :],
                                         lv["a3"][64:66, cs], start=True,
                                         stop=False)
                        nc.tensor.matmul(psum_gin[:], ident[:], q2[:],
                                         start=False, stop=True)
                        nt = gp.tile([128, F], HD, name="nt", tag="nt")
                        nc.scalar.activation(nt[:], psum_gin[:], AF.Tanh)
                        d = gp.tile([128, F], HD, name="d", tag="d")
                        nc.vector.tensor_tensor(d[:], lv["hs"], nt[:], op=OP.subtract)
                        # e on the otherwise-idle GPSIMD; h' lands one slot
                        # later so the Pool->DVE hop never stalls DVE's queue
                        e = gp.tile([128, F], HD, name="e", tag="e")
                        nc.gpsimd.tensor_tensor(e[:], lv["rz"][:, F:2 * F], d[:],
                                                op=OP.mult)
                        lv["nt"] = nt
                        lv["e"] = e
            # ---- post-loop heads: c-outer so each chunk's finalize
            # (ACT straight from PSUM + out-DMA) overlaps the next chunk's
            # 24-matmul accumulation stream ----
            with tc.tile_pool(name="phd", bufs=2, space="PSUM") as phd, \
                 tc.tile_pool(name="fin", bufs=2) as fin:
                for c in range(NF):
                    cs = slice(c * F, (c + 1) * F)
                    ps = phd.tile([m_head, F], F32, name="pshd", tag="pshd")
                    for t in range(t_steps):
                        nc.tensor.matmul(
                            ps[:],
                            wmsx_t[:, t * m_head:(t + 1) * m_head],
                            hst[t + 1][:, cs],
                            start=(t == 0), stop=(t == t_steps - 1),
                        )
                    mu_c = fin.tile([2 * t_steps, F], F32, name="muc", tag="muc")
                    std_c = fin.tile([2 * t_steps, F], F32, name="stdc", tag="stdc")
                    nc.scalar.activation(mu_c[:], ps[0:2 * t_steps, :],
                                         AF.Identity, bias=bmu_t[:])
                    nc.sync.dma_start(omu_d[:, cs], mu_c[:])
                    nc.scalar.activation(std_c[:],
                                         ps[std_off:std_off + 2 * t_steps, :],
                                         AF.Exp, bias=bstd_t[:], scale=0.5)
                    nc.sync.dma_start(ostd_d[:, cs], std_c[:])

    nc.compile()
    return nc


_NC_CACHE = {}


def _get_nc(debug=False):
    if "nc" not in _NC_CACHE:
        _NC_CACHE["nc"] = build_nc(debug=debug)
    return _NC_CACHE["nc"]


def make_in_maps(last_obs_state, enc_h_feat, z, sg, fut_traj,
                 W_dh, b_dh, W_vel, b_vel, W_ih, b_ih, W_hh, b_hh,
                 W_mu, b_mu, W_std, b_std, t_steps=T):
    f32 = np.float32
    f16 = np.float16

    # ---- weight packing (core-independent) ----
    # W_big: (KIN, 512) ; out cols = [gi_r, gi_z, gi_n, h0]
    wbig = np.zeros((KIN, 512), f32)
    wbig[0:1056, 0:384] = W_ih[:, 0:1056].T
    wbig[0:1056, 384:512] = W_dh.T
    # sg rows: rel = (sg - lo[:, :2])/dt feeds W_ih[:, 1058:1060]
    wbig[1056:1058, 0:384] = (W_ih[:, 1058:1060] / DT_CONST).T
    # lo rows (6): first two carry -W_rel/dt
    wbig[1058:1060, 0:384] = (-W_ih[:, 1058:1060] / DT_CONST).T
    # ones row: input-side biases
    wbig[1064, 0:384] = b_ih
    wbig[1064, 384:512] = b_dh

    whht = np.ascontiguousarray(W_hh.T).astype(f16)          # (128, 384)
    k3p = np.zeros((66, H), f32)
    k3p[0:2, 0:H] = W_ih[0:128, 1056:1058].T                 # a -> r gate
    k3p[2, 0:H] = b_hh[0:128]
    k3p[32:34] = W_ih[128:256, 1056:1058].T                  # a -> z gate
    k3p[34] = b_hh[128:256]
    k3p[64:66] = W_ih[256:384, 1056:1058].T                  # a -> n gate
    k3p = k3p.astype(f16)
    # head lhsT variants: variant t scatters W_mu/W_std columns to output
    # rows {t, T+t, 2T+t, 3T+t}
    std_off = ((2 * t_steps + 31) // 32) * 32
    m_head = std_off + 2 * t_steps
    wmsx = np.zeros((H, t_steps, m_head), f32)
    for t in range(t_steps):
        wmsx[:, t, t] = W_mu[0]
        wmsx[:, t, t_steps + t] = W_mu[1]
        wmsx[:, t, std_off + t] = W_std[0]
        wmsx[:, t, std_off + t_steps + t] = W_std[1]
    wmsx = wmsx.reshape(H, t_steps * m_head).astype(f16)
    bhhn = b_hh[256:384].reshape(H, 1).astype(f32)
    bmu48 = np.repeat(b_mu, t_steps).reshape(2 * t_steps, 1).astype(f32)
    bstd48 = 0.5 * np.repeat(b_std, t_steps).reshape(2 * t_steps, 1).astype(f32)

    identh = np.eye(H, dtype=f16)
    # host-side tiny matmul for a0 (0.4 MFLOP)
    a0 = last_obs_state @ W_vel.T + b_vel                    # (B, 2)

    import ml_dtypes
    f8 = ml_dtypes.float8_e4m3
    whi = wbig.astype(np.float32) * 32.0
    w8h = whi.astype(f8)
    w8l = (whi - w8h.astype(np.float32)).astype(f8)
    # DoubleRow pairing: chunk q pairs rows 256q+p and 256q+128+p
    w8h_p = w8h[0:1024].reshape(4, 2, 128, 512).transpose(0, 2, 1, 3).copy()
    w8l_p = w8l[0:1024].reshape(4, 2, 128, 512).transpose(0, 2, 1, 3).copy()
    wr = (wbig[1024:1065] * 32.0).astype(f16)

    in_maps = []
    for c in range(NCORES):
        sl = slice(c * BC, (c + 1) * BC)
        xt = np.empty((KIN, BC), f32)
        xt[0:MLP] = enc_h_feat[sl].T
        xt[MLP:1056] = z[sl].T
        xt[1056:1058] = sg[sl].T
        xt[1058:1064] = last_obs_state[sl].T
        xt[1064] = 1.0
        x8h = xt[0:1024].astype(f8)
        x8l = (xt[0:1024] - x8h.astype(np.float32)).astype(f8)
        x8h_p = x8h.reshape(4, 2, 128, BC).transpose(0, 2, 1, 3).copy()
        x8l_p = x8l.reshape(4, 2, 128, BC).transpose(0, 2, 1, 3).copy()
        a3 = np.empty((t_steps, 3, BC), f32)
        a3[0, 0:2] = a0[sl].T
        for t in range(1, t_steps):
            a3[t, 0:2] = fut_traj[t - 1, sl, 2:4].T
        a3[:, 2] = 1.0
        in_maps.append({
            "x8h": x8h_p, "x8l": x8l_p, "w8h": w8h_p, "w8l": w8l_p,
            "xtr": xt[1024:1065].astype(f16), "wr": wr,
            "a3": a3.astype(f16),
            "whht": whht,
            "k3p": k3p,
            "wmsx": wmsx,
            "identh": identh,
            "bhhn": bhhn,
            "bmu48": bmu48,
            "bstd48": bstd48,
        })
    return in_maps


def unpack_outputs(results, t_steps=T):
    mus = np.empty((t_steps, B, 2), np.float32)
    stds = np.empty((t_steps, B, 2), np.float32)
    for c in range(NCORES):
        sl = slice(c * BC, (c + 1) * BC)
        omu = results[c]["omu"].reshape(2, t_steps, BC)
        ostd = results[c]["ostd"].reshape(2, t_steps, BC)
        mus[:, sl, 0] = omu[0]
        mus[:, sl, 1] = omu[1]
        stds[:, sl, 0] = ostd[0]
        stds[:, sl, 1] = ostd[1]
    return mus, stds


def kernel(last_obs_state, enc_h_feat, z, sg, fut_traj,
           W_dh, b_dh, W_vel, b_vel, W_ih, b_ih, W_hh, b_hh,
           W_mu, b_mu, W_std, b_std):
    args = dict(
        last_obs_state=np.asarray(last_obs_state, np.float32),
        enc_h_feat=np.asarray(enc_h_feat, np.float32),
        z=np.asarray(z, np.float32),
        sg=np.asarray(sg, np.float32),
        fut_traj=np.asarray(fut_traj, np.float32),
        W_dh=np.asarray(W_dh, np.float32), b_dh=np.asarray(b_dh, np.float32),
        W_vel=np.asarray(W_vel, np.float32), b_vel=np.asarray(b_vel, np.float32),
        W_ih=np.asarray(W_ih, np.float32), b_ih=np.asarray(b_ih, np.float32),
        W_hh=np.asarray(W_hh, np.float32), b_hh=np.asarray(b_hh, np.float32),
        W_mu=np.asarray(W_mu, np.float32), b_mu=np.asarray(b_mu, np.float32),
        W_std=np.asarray(W_std, np.float32), b_std=np.asarray(b_std, np.float32),
    )
    nc = _get_nc()
    in_maps = make_in_maps(**args)
    res = run_bass_kernel_spmd(nc, in_maps, core_ids=list(range(NCORES)))
    return unpack_outputs(res.results)


# revision 47
# speedup vs baseline: 1.0100x; 1.0100x over previous
"""Trainium2 Bass/Tile kernel for nn_Decoder (GRU decoder with teacher forcing).

Math (per reference):
  zx  = [enc_h_feat, z]                    (B, 1056)
  h0  = zx @ W_dh.T + b_dh                 (B, 128)
  a0  = last_obs @ W_vel.T + b_vel         (B, 2)
  rel = (sg - last_obs[:, :2]) / dt        (B, 2)
  a_t = a0 if t==0 else fut_traj[t-1,:,2:4]
  x_t = [zx, a_t, rel]  -> GRUCell(x_t, h) -> mu_t, std_t

Device strategy (8 cores, batch-sharded, 2048 rows/core), all fp16
operands with fp32 PSUM accumulation:
  - Feature-on-partition, batch-on-free layout; free chunks of 512.
  - Setup: [gi_r|gi_z|gi_n|h0](512 rows) = W_big.T @ XT with K=1065
    host-packed rows [zxT; sgT; loT; ones].  The rel term and all
    input-side biases are folded into W_big on the host.  The K=1024 zx
    block runs as fp8e4m3 DoubleRow matmuls (256 K-rows/instruction at
    0.5 cy/row) with hi+lo residual splits of both operands, 3 passes
    (hi*whi + lo*whi + hi*wlo); the 41-row tail stays fp16.  W is
    host-scaled x32 into e4m3's normal range (raw |W|~0.03 lands in
    fp8 subnormals, which cost 13x in end-to-end error) and the PSUM
    readout divides it back out.
  - Per step/chunk i=(t,c), software-pipelined one chunk deep so PE's
    in-order stream never waits on the sigmoid->q chain:
      stage A(i):  psum_rz  = Whh_{r,z}@h + K3@[a;1] + I@gi_{r,z}   [PE x6]
                   psum_hn  = Whh_n@h                               [PE x1]
                   rz       = sigmoid(psum_rz)                      [ScalarE]
                   q        = (psum_hn + b_hh_n) * r                [DVE stt]
      stage B(i-1): q2 = q + gi_n                     [DVE fp16 2x tt]
                   psum_gin = K2@a + I@q2                           [PE x2]
                   n  = tanh(psum_gin)                              [ScalarE]
                   d = h - n   [DVE 2x tt];   e = z*d       [GPSIMD]
      stage C(i-2): h' = n + e                        [DVE fp16 2x tt]
    (9 PE matmuls per step-chunk is the floor: every psum-accumulated
    term costs one 512-cycle stream regardless of K; a DVE->PSUM
    warm-start that would drop I@q2 computes wrong results on HW.)
    h' for every step is kept in SBUF (25 x 0.5MB fp16 tiles), so no
    PSUM bank is pinned during the loop: rz/hn/gin pools all run bufs=2
    and PE never stalls on a WAR against the activation reads.
  - Post-loop: per-step head matmuls with a scattered-column lhsT
    accumulate mu/std pre-activations for ALL steps into 4 PSUM tiles;
    mu = Identity(+b_mu), std = Exp(0.5*(.)+0.5*b_std) = sqrt(exp(.)).
  - Engine balance per step-chunk in the loop: PE 1.92us (bottleneck), ScalarE
    1.65us, DVE 1.64us, GPSIMD 1.1us.  Bulk DMAs issue from SP/
    Activation sequencers (HWDGE path, no engine cost); setup weights
    ride GPSIMD's SWDGE queue, which runs parallel to HWDGE.
Host does only sharding/transposes/weight packing (a0 is a (B,6)@(6,2)
matmul on host, ~0.4 MFLOP, negligible vs the 52 GFLOP kernel).
"""

import numpy as np

import concourse.bass as bass
import concourse.mybir as mybir
import concourse.tile as tile
from concourse import bacc
from concourse.bass_utils import run_bass_kernel_spmd

F32 = mybir.dt.float32
F16 = mybir.dt.float16
F8 = mybir.dt.float8e4
DRM = mybir.MatmulPerfMode.DoubleRow
AF = mybir.ActivationFunctionType
OP = mybir.AluOpType

B, T, MLP, ZD, H, NS, NP = 16384, 24, 1024, 32, 128, 6, 2
NCORES = 8
BC = B // NCORES            # 2048 rows per core
F = 512                     # free-dim chunk
NF = BC // F                # 4 chunks
KIN = MLP + ZD + NP + NS + 1  # 1065 = zx(1056) + sg(2) + lo(6) + ones(1)
NKC = (KIN + 127) // 128    # 9 K-chunks (8x128 + 41)
DT_CONST = 0.4 * 12


def build_nc(debug=False, t_steps=T):
    HD = F16
    nc = bacc.Bacc("TRN2", target_bir_lowering=False, debug=debug)

    # ---- DRAM I/O ----
    # setup operands: 4 chunks of 256 K-rows as fp8 hi/lo (DoubleRow pairs
    # rows p and p+128 of a chunk), plus a 41-row fp16 remainder
    x8_d = nc.dram_tensor("x8", [4, 128, 4, BC], F8, kind="ExternalInput").ap()
    w8_d = nc.dram_tensor("w8", [4, 128, 4, 512], F8, kind="ExternalInput").ap()
    xtr_d = nc.dram_tensor("xtr", [41, BC], F16, kind="ExternalInput").ap()
    wr_d = nc.dram_tensor("wr", [41, 512], F16, kind="ExternalInput").ap()
    a3_d = nc.dram_tensor("a3", [t_steps, 3, BC], F16, kind="ExternalInput").ap()
    whht_d = nc.dram_tensor("whht", [H, 3 * H], F16, kind="ExternalInput").ap()
    k3p_d = nc.dram_tensor("k3p", [66, H], F16, kind="ExternalInput").ap()
    _std_off = ((2 * t_steps + 31) // 32) * 32
    _m_head = _std_off + 2 * t_steps
    wmsx_d = nc.dram_tensor("wmsx", [H, t_steps * _m_head], F16,
                            kind="ExternalInput").ap()
    id_d = nc.dram_tensor("identh", [H, H], F16, kind="ExternalInput").ap()
    bhhn_d = nc.dram_tensor("bhhn", [H, 1], F32, kind="ExternalInput").ap()
    bmu_d = nc.dram_tensor("bmu48", [2 * t_steps, 1], F32, kind="ExternalInput").ap()
    bstd_d = nc.dram_tensor("bstd48", [2 * t_steps, 1], F32, kind="ExternalInput").ap()
    omu_d = nc.dram_tensor("omu", [2 * t_steps, BC], F32, kind="ExternalOutput").ap()
    ostd_d = nc.dram_tensor("ostd", [2 * t_steps, BC], F32, kind="ExternalOutput").ap()

    with tile.TileContext(nc) as tc:
        with tc.tile_pool(name="persist", bufs=1) as pp:
            # persistent SBUF state
            gi_r = pp.tile([H, BC], HD)
            gi_z = pp.tile([H, BC], HD)
            gi_n = pp.tile([H, BC], HD)
            # h history: one tile per step boundary (h_state[t] = h before
            # step t); separate tiles keep dependency tracking per-step
            hst = [pp.tile([H, BC], HD, name=f"hst{t}") for t in range(t_steps + 1)]
            std_off = ((2 * t_steps + 31) // 32) * 32
            m_head = std_off + 2 * t_steps
            whht_t = pp.tile([H, 3 * H], HD)
            k3p_t = pp.tile([66, H], HD)
            wmsx_t = pp.tile([H, t_steps * m_head], HD)
            bhhn_t = pp.tile([H, 1], F32)
            bmu_t = pp.tile([2 * t_steps, 1], F32)
            bstd_t = pp.tile([2 * t_steps, 1], F32)
            ident = pp.tile([H, H], HD)

            gi_dst = [gi_r, gi_z, gi_n, None]

            # ---- setup: [gi | h0] = W_big.T @ XT ----
            # fp8 DoubleRow 3-pass (hi*whi + lo*whi + hi*wlo): 0.5 cy/row
            # and 256 K-rows per instruction; dropped lo*wlo term ~0.4%
            with tc.tile_pool(name="xtp", bufs=1) as xtp, \
                 tc.tile_pool(name="wp", bufs=1) as wp, \
                 tc.tile_pool(name="sps", bufs=4, space="PSUM") as sps:
                # hi and lo planes packed in one tile per chunk: one DMA
                # per (q, column) instead of two -- the 625ns HWDGE hold per
                # DMA made the per-column load chain outrun PE's sweep
                x8_t, w8_t = [], []
                for q in range(4):
                    x8_t.append(xtp.tile([128, 4, BC], F8, name=f"x8{q}",
                                         tag=f"x8{q}"))
                    w8_t.append(wp.tile([128, 4, 512], F8, name=f"w8{q}",
                                        tag=f"w8{q}"))
                xtr_t = xtp.tile([41, BC], F16, name="xtr", tag="xtr")
                wr_t = wp.tile([41, 512], F16, name="wr", tag="wr")
                for q in range(4):
                    _eng = nc.sync if q == 0 else nc.gpsimd
                    _eng.dma_start(w8_t[q][:], w8_d[q])
                nc.gpsimd.dma_start(wr_t[:], wr_d)
                # column(nci)-major x loads: the nci-outer group loop below
                # consumes column 0 while column 1 lands
                for nci in range(NF):
                    cs = slice(nci * F, (nci + 1) * F)
                    for q in range(4):
                        _eng = [nc.scalar, nc.sync][q % 2]
                        _eng.dma_start(x8_t[q][:, :, cs], x8_d[q][:, :, cs])
                    [nc.scalar, nc.sync][nci % 2].dma_start(
                        xtr_t[:, cs], xtr_d[:, cs])
                # loop/head weights ride the Pool SWDGE queue as well --
                # it is idle during setup and bypasses the shared HWDGE
                nc.gpsimd.dma_start(ident[:], id_d)
                nc.gpsimd.dma_start(whht_t[:], whht_d)
                nc.gpsimd.dma_start(k3p_t[:], k3p_d)
                nc.gpsimd.dma_start(bhhn_t[:], bhhn_d)
                nc.gpsimd.dma_start(wmsx_t[:], wmsx_d)
                nc.gpsimd.dma_start(bmu_t[:], bmu_d)
                nc.gpsimd.dma_start(bstd_t[:], bstd_d)

                for nci in range(NF):
                    cs = slice(nci * F, (nci + 1) * F)
                    for m in range(4):
                        ms = slice(128 * m, 128 * (m + 1))
                        ps = sps.tile([128, F], F32, name="setps", tag="setps")
                        for q in range(4):
                            nc.tensor.matmul(ps[:], w8_t[q][:, 0:2, ms],
                                             x8_t[q][:, 0:2, cs],
                                             start=(q == 0), stop=False,
                                             perf_mode=DRM)
                            nc.tensor.matmul(ps[:], w8_t[q][:, 0:2, ms],
                                             x8_t[q][:, 2:4, cs],
                                             start=False, stop=False,
                                             perf_mode=DRM)
                            nc.tensor.matmul(ps[:], w8_t[q][:, 2:4, ms],
                                             x8_t[q][:, 0:2, cs],
                                             start=False, stop=False,
                                             perf_mode=DRM)
                        nc.tensor.matmul(ps[:], wr_t[:, ms], xtr_t[:, cs],
                                         start=False, stop=True)
                        dst = gi_dst[m] if m < 3 else hst[0]
                        # W was host-scaled x32 into e4m3's normal range
                        # (raw |W|~0.03 sits in fp8 subnormals); undo here
                        nc.vector.tensor_scalar_mul(dst[:, cs], ps[:],
                                                    1.0 / 32.0)

            # ---- recurrence, software-pipelined one chunk deep ----
            NI = t_steps * NF
            with tc.tile_pool(name="a3p", bufs=3) as a3p, \
                 tc.tile_pool(name="gp", bufs=3) as gp, \
                 tc.tile_pool(name="prz", bufs=2, space="PSUM") as prz, \
                 tc.tile_pool(name="phn", bufs=2, space="PSUM") as phn, \
                 tc.tile_pool(name="pgin", bufs=2, space="PSUM") as pgin:
                a3_tiles = {}
                live = {}   # flat index -> dict of tiles for stage B
                for i in range(NI + 2):
                    # ---- stage C for flat index i-2 (emitted first so the
                    # h' write clears DVE's in-order queue before stt/q2/d
                    # pile in; the next step's matmuls wait on it) ----
                    if i >= 2:
                        lv = live.pop(i - 2)
                        nc.vector.tensor_tensor(hst[lv["t"] + 1][:, lv["cs"]],
                                                lv["nt"][:], lv["e"][:], op=OP.add)
                    # ---- stage A for flat index i ----
                    if i < NI:
                        t, c = divmod(i, NF)
                        cs = slice(c * F, (c + 1) * F)
                        hs = hst[t][:, cs]
                        if c == 0:
                            a3_t = a3p.tile([66, BC], HD, name="a3t", tag="a3t")
                            nc.sync.dma_start(a3_t[0:3, :], a3_d[t])
                            nc.sync.dma_start(a3_t[32:35, :], a3_d[t])
                            nc.sync.dma_start(a3_t[64:66, :], a3_d[t][0:2])
                            a3_tiles[t] = a3_t
                        a3_t = a3_tiles[t]
                        psum_rz = prz.tile([128, 2 * F], F32, name="psrz", tag="psrz")
                        nc.tensor.matmul(psum_rz[:, 0:F], whht_t[:, 0:H], hs,
                                         start=True, stop=False)
                        nc.tensor.matmul(psum_rz[:, F:2 * F], whht_t[:, H:2 * H], hs,
                                         start=True, stop=False)
                        nc.tensor.matmul(psum_rz[:, 0:F], k3p_t[0:3, :],
                                         a3_t[0:3, cs], start=False, stop=False)
                        nc.tensor.matmul(psum_rz[:, F:2 * F], k3p_t[32:35, :],
                                         a3_t[32:35, cs], start=False, stop=False)
                        nc.tensor.matmul(psum_rz[:, 0:F], ident[:], gi_r[:, cs],
                                         start=False, stop=True)
                        nc.tensor.matmul(psum_rz[:, F:2 * F], ident[:], gi_z[:, cs],
                                         start=False, stop=True)
                        psum_hn = phn.tile([128, F], F32, name="pshn", tag="pshn")
                        nc.tensor.matmul(psum_hn[:], whht_t[:, 2 * H:3 * H], hs,
                                         start=True, stop=True)
                        rz = gp.tile([128, 2 * F], HD, name="rz", tag="rz")
                        nc.scalar.activation(rz[:], psum_rz[:], AF.Sigmoid)
                        q = gp.tile([128, F], HD, name="q", tag="q")
                        nc.vector.scalar_tensor_tensor(
                            q[:], psum_hn[:], bhhn_t[:], rz[:, 0:F],
                            op0=OP.add, op1=OP.mult,
                        )
                        live[i] = dict(t=t, cs=cs, hs=hs, rz=rz, a3=a3_t, q=q)
                    # ---- stage B for flat index i-1 ----
                    if 1 <= i <= NI:
                        lv = live[i - 1]
                        t, cs = lv["t"], lv["cs"]
                        # q2 = q + gi_n on DVE replaces a third PE inject
                        q2 = gp.tile([128, F], HD, name="q2", tag="q2")
                        nc.vector.tensor_tensor(q2[:], lv["q"][:], gi_n[:, cs],
                                                op=OP.add)
                        psum_gin = pgin.tile([128, F], F32, name="psgin",
                                             tag="psgin")
                        nc.tensor.matmul(psum_gin[:], k3p_t[64:66, :],
                                         lv["a3"][64:66, cs], start=True,
                                         stop=False)
                        nc.tensor.matmul(psum_gin[:], ident[:], q2[:],
                                         start=False, stop=True)
                        nt = gp.tile([128, F], HD, name="nt", tag="nt")
                        nc.scalar.activation(nt[:], psum_gin[:], AF.Tanh)
                        d = gp.tile([128, F], HD, name="d", tag="d")
                        nc.vector.tensor_tensor(d[:], lv["hs"], nt[:], op=OP.subtract)
                        # e on the otherwise-idle GPSIMD; h' lands one slot
                        # later so the Pool->DVE hop never stalls DVE's queue
                        e = gp.tile([128, F], HD, name="e", tag="e")
                        nc.gpsimd.tensor_tensor(e[:], lv["rz"][:, F:2 * F], d[:],
                                                op=OP.mult)
                        lv["nt"] = nt
                        lv["e"] = e
            # ---- post-loop heads: c-outer so each chunk's finalize
            # (ACT straight from PSUM + out-DMA) overlaps the next chunk's
            # 24-matmul accumulation stream ----
            with tc.tile_pool(name="phd", bufs=2, space="PSUM") as phd, \
                 tc.tile_pool(name="fin", bufs=2) as fin:
                for c in range(NF):
                    cs = slice(c * F, (c + 1) * F)
                    ps = phd.tile([m_head, F], F32, name="pshd", tag="pshd")
                    for t in range(t_steps):
                        nc.tensor.matmul(
                            ps[:],
                            wmsx_t[:, t * m_head:(t + 1) * m_head],
                            hst[t + 1][:, cs],
                            start=(t == 0), stop=(t == t_steps - 1),
                        )
                    mu_c = fin.tile([2 * t_steps, F], F32, name="muc", tag="muc")
                    std_c = fin.tile([2 * t_steps, F], F32, name="stdc", tag="stdc")
                    nc.scalar.activation(mu_c[:], ps[0:2 * t_steps, :],
                                         AF.Identity, bias=bmu_t[:])
                    nc.sync.dma_start(omu_d[:, cs], mu_c[:])
                    nc.scalar.activation(std_c[:],
                                         ps[std_off:std_off + 2 * t_steps, :],
                                         AF.Exp, bias=bstd_t[:], scale=0.5)
                    nc.sync.dma_start(ostd_d[:, cs], std_c[:])

    nc.compile()
    return nc


_NC_CACHE = {}


def _get_nc(debug=False):
    if "nc" not in _NC_CACHE:
        _NC_CACHE["nc"] = build_nc(debug=debug)
    return _NC_CACHE["nc"]


def make_in_maps(last_obs_state, enc_h_feat, z, sg, fut_traj,
                 W_dh, b_dh, W_vel, b_vel, W_ih, b_ih, W_hh, b_hh,
                 W_mu, b_mu, W_std, b_std, t_steps=T):
    f32 = np.float32
    f16 = np.float16

    # ---- weight packing (core-independent) ----
    # W_big: (KIN, 512) ; out cols = [gi_r, gi_z, gi_n, h0]
    wbig = np.zeros((KIN, 512), f32)
    wbig[0:1056, 0:384] = W_ih[:, 0:1056].T
    wbig[0:1056, 384:512] = W_dh.T
    # sg rows: rel = (sg - lo[:, :2])/dt feeds W_ih[:, 1058:1060]
    wbig[1056:1058, 0:384] = (W_ih[:, 1058:1060] / DT_CONST).T
    # lo rows (6): first two carry -W_rel/dt
    wbig[1058:1060, 0:384] = (-W_ih[:, 1058:1060] / DT_CONST).T
    # ones row: input-side biases
    wbig[1064, 0:384] = b_ih
    wbig[1064, 384:512] = b_dh

    whht = np.ascontiguousarray(W_hh.T).astype(f16)          # (128, 384)
    k3p = np.zeros((66, H), f32)
    k3p[0:2, 0:H] = W_ih[0:128, 1056:1058].T                 # a -> r gate
    k3p[2, 0:H] = b_hh[0:128]
    k3p[32:34] = W_ih[128:256, 1056:1058].T                  # a -> z gate
    k3p[34] = b_hh[128:256]
    k3p[64:66] = W_ih[256:384, 1056:1058].T                  # a -> n gate
    k3p = k3p.astype(f16)
    # head lhsT variants: variant t scatters W_mu/W_std columns to output
    # rows {t, T+t, 2T+t, 3T+t}
    std_off = ((2 * t_steps + 31) // 32) * 32
    m_head = std_off + 2 * t_steps
    wmsx = np.zeros((H, t_steps, m_head), f32)
    for t in range(t_steps):
        wmsx[:, t, t] = W_mu[0]
        wmsx[:, t, t_steps + t] = W_mu[1]
        wmsx[:, t, std_off + t] = W_std[0]
        wmsx[:, t, std_off + t_steps + t] = W_std[1]
    wmsx = wmsx.reshape(H, t_steps * m_head).astype(f16)
    bhhn = b_hh[256:384].reshape(H, 1).astype(f32)
    bmu48 = np.repeat(b_mu, t_steps).reshape(2 * t_steps, 1).astype(f32)
    bstd48 = 0.5 * np.repeat(b_std, t_steps).reshape(2 * t_steps, 1).astype(f32)

    identh = np.eye(H, dtype=f16)
    # host-side tiny matmul for a0 (0.4 MFLOP)
    a0 = last_obs_state @ W_vel.T + b_vel                    # (B, 2)

    import ml_dtypes
    f8 = ml_dtypes.float8_e4m3
    whi = wbig.astype(np.float32) * 32.0
    w8h = whi.astype(f8)
    w8l = (whi - w8h.astype(np.float32)).astype(f8)
    # DoubleRow pairing: chunk q pairs rows 256q+p and 256q+128+p
    w8h_p = w8h[0:1024].reshape(4, 2, 128, 512).transpose(0, 2, 1, 3)
    w8l_p = w8l[0:1024].reshape(4, 2, 128, 512).transpose(0, 2, 1, 3)
    w8_p = np.concatenate([w8h_p, w8l_p], axis=2).copy()
    wr = (wbig[1024:1065] * 32.0).astype(f16)

    in_maps = []
    for c in range(NCORES):
        sl = slice(c * BC, (c + 1) * BC)
        xt = np.empty((KIN, BC), f32)
        xt[0:MLP] = enc_h_feat[sl].T
        xt[MLP:1056] = z[sl].T
        xt[1056:1058] = sg[sl].T
        xt[1058:1064] = last_obs_state[sl].T
        xt[1064] = 1.0
        x8h = xt[0:1024].astype(f8)
        x8l = (xt[0:1024] - x8h.astype(np.float32)).astype(f8)
        x8h_p = x8h.reshape(4, 2, 128, BC).transpose(0, 2, 1, 3)
        x8l_p = x8l.reshape(4, 2, 128, BC).transpose(0, 2, 1, 3)
        x8_p = np.concatenate([x8h_p, x8l_p], axis=2).copy()
        a3 = np.empty((t_steps, 3, BC), f32)
        a3[0, 0:2] = a0[sl].T
        for t in range(1, t_steps):
            a3[t, 0:2] = fut_traj[t - 1, sl, 2:4].T
        a3[:, 2] = 1.0
        in_maps.append({
            "x8": x8_p, "w8": w8_p,
            "xtr": xt[1024:1065].astype(f16), "wr": wr,
            "a3": a3.astype(f16),
            "whht": whht,
            "k3p": k3p,
            "wmsx": wmsx,
            "identh": identh,
            "bhhn": bhhn,
            "bmu48": bmu48,
            "bstd48": bstd48,
        })
    return in_maps


def unpack_outputs(results, t_steps=T):
    mus = np.empty((t_steps, B, 2), np.float32)
    stds = np.empty((t_steps, B, 2), np.float32)
    for c in range(NCORES):
        sl = slice(c * BC, (c + 1) * BC)
        omu = results[c]["omu"].reshape(2, t_steps, BC)
        ostd = results[c]["ostd"].reshape(2, t_steps, BC)
        mus[:, sl, 0] = omu[0]
        mus[:, sl, 1] = omu[1]
        stds[:, sl, 0] = ostd[0]
        stds[:, sl, 1] = ostd[1]
    return mus, stds


def kernel(last_obs_state, enc_h_feat, z, sg, fut_traj,
           W_dh, b_dh, W_vel, b_vel, W_ih, b_ih, W_hh, b_hh,
           W_mu, b_mu, W_std, b_std):
    args = dict(
        last_obs_state=np.asarray(last_obs_state, np.float32),
        enc_h_feat=np.asarray(enc_h_feat, np.float32),
        z=np.asarray(z, np.float32),
        sg=np.asarray(sg, np.float32),
        fut_traj=np.asarray(fut_traj, np.float32),
        W_dh=np.asarray(W_dh, np.float32), b_dh=np.asarray(b_dh, np.float32),
        W_vel=np.asarray(W_vel, np.float32), b_vel=np.asarray(b_vel, np.float32),
        W_ih=np.asarray(W_ih, np.float32), b_ih=np.asarray(b_ih, np.float32),
        W_hh=np.asarray(W_hh, np.float32), b_hh=np.asarray(b_hh, np.float32),
        W_mu=np.asarray(W_mu, np.float32), b_mu=np.asarray(b_mu, np.float32),
        W_std=np.asarray(W_std, np.float32), b_std=np.asarray(b_std, np.float32),
    )
    nc = _get_nc()
    in_maps = make_in_maps(**args)
    res = run_bass_kernel_spmd(nc, in_maps, core_ids=list(range(NCORES)))
    return unpack_outputs(res.results)
